# revision 6
# baseline (speedup 1.0000x reference)
"""Trainium2 Bass kernel computing out = x * exp(diagonal).

x: (8192, 4096) float32, diagonal: (4096,) float32.
Data-parallel across 8 NeuronCores: each core handles 1024 rows of x;
the 4096-float diagonal is replicated to every core.

Per-core program (pure streaming; the 16 SDMA engines aggregate
~435 GB/s and bound the kernel, so 32 MiB of x in+out traffic floors
at ~78 us; everything else must hide under that):

  1. exp(diagonal) broadcast tile [128, 4096] built from a stride-0
     SWDGE DMA (DRAM -> all partitions) issued FIRST, before any x
     load, so it streams uncontended (~5 us) and ACT's exp is ready
     before the first x tile lands. A 1-element DVE copy observes the
     Exp so the muls below carry exactly one wait (their own load).
  2. x streams through 4 fresh [128, 2, 4096] SBUF tiles with the two
     row-folds CONSECUTIVE in DRAM ("(s p n) m" layout): each DMA
     descriptor covers 32 KiB contiguous, halving descriptor-fetch
     pressure on the SDMA engines (engine 15 shares its AXI port with
     descriptor traffic and is the straggler otherwise).
  3. Each loaded tile is multiplied and stored in two [128, 1, 4096]
     halves so stores enter the DMA queues as early as possible.
"""

import numpy as np

BATCH, FEAT = 8192, 4096
N_CORES = 8
ROWS = BATCH // N_CORES   # 1024 rows per core
P = 128                   # SBUF partitions
FOLD = 2                  # consecutive DRAM rows folded into one partition
N_TILES = ROWS // (P * FOLD)  # 4 tiles of [128, 2, 4096] per core

_CACHE = {}


def build_nc(rows=ROWS, feat=FEAT, fold=FOLD):
    import concourse.bacc as bacc
    import concourse.mybir as mybir
    from concourse import tile

    # Bacc (not plain Bass): its compile() pass splits multi-sem waits into
    # EventSemaphore chains -- TRN2 instructions carry at most one wait.
    nc = bacc.Bacc("TRN2", target_bir_lowering=False, debug=False)
    x = nc.dram_tensor("x", (rows, feat), mybir.dt.float32, kind="ExternalInput").ap()
    d = nc.dram_tensor("d", (feat,), mybir.dt.float32, kind="ExternalInput").ap()
    out = nc.dram_tensor(
        "out", (rows, feat), mybir.dt.float32, kind="ExternalOutput"
    ).ap()

    n_tiles = rows // (P * fold)
    # n INSIDE p: partition p holds `fold` consecutive DRAM rows -> one
    # 32 KiB contiguous descriptor per partition per DMA.
    x_t = x.rearrange("(s p n) m -> s p n m", p=P, n=fold)
    o_t = out.rearrange("(s p n) m -> s p n m", p=P, n=fold)
    d_row = d.rearrange("(r c) -> r c", r=1)

    with tile.TileContext(nc) as tc:
        with (
            tc.tile_pool(name="const", bufs=1) as cpool,
            tc.tile_pool(name="io", bufs=n_tiles) as iopool,
        ):
            expd = cpool.tile([P, feat], mybir.dt.float32)
            # Stride-0 broadcast from DRAM, issued before any x load.
            nc.gpsimd.dma_start(expd[:], d_row.to_broadcast((P, feat)))
            nc.scalar.activation(expd[:], expd[:], mybir.ActivationFunctionType.Exp)
            # DVE observer: absorbs the wait on the Exp so the muls below
            # carry exactly one wait (their own load DMA).
            scratch = cpool.tile([1, 1], mybir.dt.float32)
            nc.vector.tensor_copy(scratch[:], expd[0:1, 0:1])
            expd3 = expd[:].rearrange("p (o m) -> p o m", o=1)

            tiles = []
            for i in range(n_tiles):
                t = iopool.tile([P, fold * feat], mybir.dt.float32)
                t3 = t.rearrange("p (n m) -> p n m", n=fold)
                nc.sync.dma_start(t3, x_t[i])
                tiles.append(t3)
            for i, t3 in enumerate(tiles):
                for j in range(fold):
                    nc.vector.tensor_mul(t3[:, j : j + 1], t3[:, j : j + 1], expd3)
                    nc.scalar.dma_start(o_t[i, :, j : j + 1], t3[:, j : j + 1])
    nc.finalize()
    return nc


def kernel(x, diagonal):
    from concourse.bass_utils import run_bass_kernel_spmd

    if "nc" not in _CACHE:
        _CACHE["nc"] = build_nc()
    nc = _CACHE["nc"]

    x = np.ascontiguousarray(x, dtype=np.float32)
    d = np.ascontiguousarray(diagonal, dtype=np.float32)
    in_maps = [{"x": x[c * ROWS : (c + 1) * ROWS], "d": d} for c in range(N_CORES)]
    res = run_bass_kernel_spmd(nc, in_maps, core_ids=list(range(N_CORES)))
    _CACHE["last_res"] = res
    return np.concatenate([r["out"] for r in res.results], axis=0)


# revision 7
# speedup vs baseline: 1.0050x; 1.0050x over previous
"""Trainium2 Bass kernel computing out = x * exp(diagonal).

x: (8192, 4096) float32, diagonal: (4096,) float32.
Data-parallel across 8 NeuronCores: each core handles 1024 rows of x;
the diagonal is replicated to every core.

Per-core program (pure streaming; the 16 SDMA engines aggregate
~435 GB/s and bound the kernel, so 32 MiB of x in+out traffic floors
at ~78 us; everything else must hide under that):

  1. The host replicates the 16 KiB diagonal to a (128, 4096) array, so
     the on-device partition broadcast is a plain full-rate 2 MiB load
     (a stride-0 DRAM broadcast re-reads one 16 KiB page 128x and is
     single-HBM-channel-bound at ~25 us; a replicated source spreads
     across channels and takes ~5 us). Issued FIRST on the SP queue so
     it completes before the first x tile; ACT then exps it in place.
     A 1-element DVE copy observes the Exp so the muls below carry
     exactly one wait (their own load DMA).
  2. x streams through 4 fresh [128, 2, 4096] SBUF tiles with the two
     row-folds CONSECUTIVE in DRAM ("(s p n) m" layout): each DMA
     descriptor covers 32 KiB contiguous, halving descriptor-fetch
     pressure on the SDMA engines (engine 15 shares its AXI port with
     descriptor traffic and is the straggler otherwise).
  3. Each loaded tile is multiplied and stored in two [128, 1, 4096]
     halves so stores enter the DMA queues as early as possible.
"""

import numpy as np

BATCH, FEAT = 8192, 4096
N_CORES = 8
ROWS = BATCH // N_CORES   # 1024 rows per core
P = 128                   # SBUF partitions
FOLD = 2                  # consecutive DRAM rows folded into one partition
N_TILES = ROWS // (P * FOLD)  # 4 tiles of [128, 2, 4096] per core

_CACHE = {}


def build_nc(rows=ROWS, feat=FEAT, fold=FOLD):
    import concourse.bacc as bacc
    import concourse.mybir as mybir
    from concourse import tile

    # Bacc (not plain Bass): its compile() pass splits multi-sem waits into
    # EventSemaphore chains -- TRN2 instructions carry at most one wait.
    nc = bacc.Bacc("TRN2", target_bir_lowering=False, debug=False)
    x = nc.dram_tensor("x", (rows, feat), mybir.dt.float32, kind="ExternalInput").ap()
    d = nc.dram_tensor(
        "d", (P, feat), mybir.dt.float32, kind="ExternalInput"
    ).ap()
    out = nc.dram_tensor(
        "out", (rows, feat), mybir.dt.float32, kind="ExternalOutput"
    ).ap()

    n_tiles = rows // (P * fold)
    # n INSIDE p: partition p holds `fold` consecutive DRAM rows -> one
    # 32 KiB contiguous descriptor per partition per DMA.
    x_t = x.rearrange("(s p n) m -> s p n m", p=P, n=fold)
    o_t = out.rearrange("(s p n) m -> s p n m", p=P, n=fold)

    with tile.TileContext(nc) as tc:
        with (
            tc.tile_pool(name="const", bufs=1) as cpool,
            tc.tile_pool(name="io", bufs=n_tiles) as iopool,
        ):
            expd = cpool.tile([P, feat], mybir.dt.float32)
            # Host-replicated diagonal: plain full-rate load, first in the
            # SP queue so it lands before x tile 0.
            nc.sync.dma_start(expd[:], d)
            nc.scalar.activation(expd[:], expd[:], mybir.ActivationFunctionType.Exp)
            # DVE observer: absorbs the wait on the Exp so the muls below
            # carry exactly one wait (their own load DMA).
            scratch = cpool.tile([1, 1], mybir.dt.float32)
            nc.vector.tensor_copy(scratch[:], expd[0:1, 0:1])
            expd3 = expd[:].rearrange("p (o m) -> p o m", o=1)

            tiles = []
            for i in range(n_tiles):
                t = iopool.tile([P, fold * feat], mybir.dt.float32)
                t3 = t.rearrange("p (n m) -> p n m", n=fold)
                nc.sync.dma_start(t3, x_t[i])
                tiles.append(t3)
            for i, t3 in enumerate(tiles):
                for j in range(fold):
                    nc.vector.tensor_mul(t3[:, j : j + 1], t3[:, j : j + 1], expd3)
                    nc.scalar.dma_start(o_t[i, :, j : j + 1], t3[:, j : j + 1])
    nc.finalize()
    return nc


def kernel(x, diagonal):
    from concourse.bass_utils import run_bass_kernel_spmd

    if "nc" not in _CACHE:
        _CACHE["nc"] = build_nc()
    nc = _CACHE["nc"]

    x = np.ascontiguousarray(x, dtype=np.float32)
    d = np.ascontiguousarray(
        np.broadcast_to(np.asarray(diagonal, dtype=np.float32), (P, FEAT))
    )
    in_maps = [{"x": x[c * ROWS : (c + 1) * ROWS], "d": d} for c in range(N_CORES)]
    res = run_bass_kernel_spmd(nc, in_maps, core_ids=list(range(N_CORES)))
    _CACHE["last_res"] = res
    return np.concatenate([r["out"] for r in res.results], axis=0)
